# revision 9
# baseline (speedup 1.0000x reference)
"""Multi-headed self-attention on 8 trn2 NeuronCores.

Sharding: 8 cores = 4 batches x 2 head-groups (8 heads / 512 hidden dims each).
Each core runs the full MHA pipeline for (batch b, head-group g):
  hqT/hkT [512, 2048] projections in bf16 (head dims on partitions, attention
  scale folded into Wq), hv [2048, 8, 65] bf16 (j on partitions, ones column
  per head so the context matmul also produces softmax denominators),
  scoresT -> exp(bf16) -> ctx+denom matmul -> reciprocal scale,
  partial out-projection [2048, 1024] in f32.
Host sums the two group partials per batch and adds bo.
Projection matmuls run in float32r (full-rate fp32), attention in bf16.
"""

import numpy as np
from contextlib import ExitStack

B, L, HID = 4, 2048, 1024
NH, HD = 16, 64
G = 2                 # head groups (tensor-parallel dim)
GH = NH // G          # 8 heads per core
GD = GH * HD          # 512 hidden dims per core
SC = HD ** -0.5

KT = HID // 128       # 8 contraction tiles
IC = L // 512         # 4 i-chunks of 512
JT = L // 128         # 16 j-tiles of 128
DM = GD // 128        # 4 head-dim tiles of 128

_CACHE = {}


def _build():
    from concourse import bacc, bass, tile, mybir

    f32 = mybir.dt.float32
    f32r = mybir.dt.float32r
    bf16 = mybir.dt.bfloat16
    AF = mybir.ActivationFunctionType
    PSUM = bass.MemorySpace.PSUM

    def r(ap):
        return ap.bitcast(f32r)

    nc = bacc.Bacc("TRN2", target_bir_lowering=False, debug=False, num_devices=8)

    qT_d = nc.dram_tensor("qT", [HID, L], f32r, kind="ExternalInput")
    kT_d = nc.dram_tensor("kT", [HID, L], f32r, kind="ExternalInput")
    vT_d = nc.dram_tensor("vT", [HID, L], f32r, kind="ExternalInput")
    wq_d = nc.dram_tensor("wqT", [HID, GD], f32r, kind="ExternalInput")
    wk_d = nc.dram_tensor("wkT", [HID, GD], f32r, kind="ExternalInput")
    wv_d = nc.dram_tensor("wvT", [HID, GD], f32r, kind="ExternalInput")
    wo_d = nc.dram_tensor("woT", [128, DM, HID], bf16, kind="ExternalInput")
    bq_d = nc.dram_tensor("bq", [128, DM], f32, kind="ExternalInput")
    bk_d = nc.dram_tensor("bk", [128, DM], f32, kind="ExternalInput")
    bv_d = nc.dram_tensor("bv", [1, GD], f32r, kind="ExternalInput")
    on_d = nc.dram_tensor("ones", [1, 128], f32r, kind="ExternalInput")
    out_d = nc.dram_tensor("out", [L, HID], f32, kind="ExternalOutput")

    with tile.TileContext(nc) as tc, ExitStack() as ctx:
        pers = ctx.enter_context(tc.tile_pool(name="pers", bufs=1))
        wpool = ctx.enter_context(tc.tile_pool(name="w", bufs=10))
        xpool = ctx.enter_context(tc.tile_pool(name="x", bufs=9))
        vpool = ctx.enter_context(tc.tile_pool(name="vt", bufs=10))
        epool = ctx.enter_context(tc.tile_pool(name="e", bufs=18))
        cxpool = ctx.enter_context(tc.tile_pool(name="cx", bufs=2))
        small = ctx.enter_context(tc.tile_pool(name="sm", bufs=2))
        opool = ctx.enter_context(tc.tile_pool(name="o", bufs=3))
        mmps = ctx.enter_context(tc.tile_pool(name="mmps", bufs=2, space=PSUM))
        sps_p = ctx.enter_context(tc.tile_pool(name="sps", bufs=3, space=PSUM))
        cps_p = ctx.enter_context(tc.tile_pool(name="cps", bufs=2, space=PSUM))

        ones = pers.tile([1, 128], f32r, tag="ones")
        nc.sync.dma_start(out=ones[:], in_=on_d[:])
        bq_s = pers.tile([128, DM], f32, tag="bq")
        nc.sync.dma_start(out=bq_s[:], in_=bq_d[:])
        bk_s = pers.tile([128, DM], f32, tag="bk")
        nc.sync.dma_start(out=bk_s[:], in_=bk_d[:])
        bv_s = pers.tile([1, GD], f32r, tag="bv")
        nc.sync.dma_start(out=bv_s[:], in_=bv_d[:])

        # persistent activations (bf16)
        hqT = pers.tile([128, DM, L], bf16, tag="hqT")      # [dim128, dm, i]
        hkT = pers.tile([128, DM, L], bf16, tag="hkT")      # [dim128, dm, j]
        hv = pers.tile([128, JT, GH, 65], bf16, tag="hv")   # [j128, jt, h, d|1]
        nc.vector.memset(hv[:, :, :, 64:65], 1.0)

        # ---- phase A: q/k projections into transposed layout ----
        def project(src_d, w_d, bias, dst):
            wts = []
            for kk in range(KT):
                wt = wpool.tile([128, GD], f32r, tag="w")
                nc.sync.dma_start(out=wt[:], in_=w_d[kk * 128:(kk + 1) * 128, :])
                wts.append(wt)
            for ic in range(IC):
                xts = []
                for kk in range(KT):
                    xt = xpool.tile([128, 512], f32r, tag="xs")
                    nc.sync.dma_start(
                        out=xt[:],
                        in_=src_d[kk * 128:(kk + 1) * 128, ic * 512:(ic + 1) * 512],
                    )
                    xts.append(xt)
                for dm in range(DM):
                    ps = mmps.tile([128, 512], f32, tag="mm")
                    for kk in range(KT):
                        nc.tensor.matmul(
                            ps[:],
                            wts[kk][:, dm * 128:(dm + 1) * 128],
                            xts[kk][:],
                            start=(kk == 0),
                            stop=(kk == KT - 1),
                        )
                    nc.vector.tensor_scalar_add(
                        dst[:, dm:dm + 1, ic * 512:(ic + 1) * 512],
                        ps[:],
                        bias[:, dm:dm + 1],
                    )

        project(qT_d, wq_d, bq_s, hqT)
        project(kT_d, wk_d, bk_s, hkT)

        # ---- v projection directly in [j, d] layout ----
        wvs = []
        for kk in range(KT):
            wt = wpool.tile([128, GD], f32r, tag="w")
            nc.sync.dma_start(out=wt[:], in_=wv_d[kk * 128:(kk + 1) * 128, :])
            wvs.append(wt)
        for jt in range(JT):
            ps = mmps.tile([128, GH, 64], f32, tag="mm")
            for kk in range(KT):
                vt = vpool.tile([128, 128], f32r, tag="vt")
                nc.sync.dma_start(
                    out=vt[:],
                    in_=vT_d[kk * 128:(kk + 1) * 128, jt * 128:(jt + 1) * 128],
                )
                nc.tensor.matmul(
                    ps[:],
                    vt[:],
                    wvs[kk][:],
                    start=(kk == 0),
                    stop=False,
                )
            nc.tensor.matmul(ps[:], ones[:], bv_s[:], start=False, stop=True)
            nc.vector.tensor_copy(hv[:, jt:jt + 1, :, 0:64], ps[:])

        # out-projection weights, bf16 (converted host-side)
        woT = pers.tile([128, DM, HID], bf16, tag="woT")
        nc.sync.dma_start(out=woT[:], in_=wo_d[:])

        # ---- phase B/C: attention + out-projection per i-chunk ----
        for ic in range(IC):
            ctxT = cxpool.tile([128, DM, 512], bf16, tag="ctxT")
            for h in range(GH):
                dm, p0 = h // 2, (h % 2) * 64
                ets = []
                for jt in range(JT):
                    sps = sps_p.tile([128, 512], f32, tag="s")
                    nc.tensor.matmul(
                        sps[:],
                        hkT[p0:p0 + 64, dm:dm + 1, jt * 128:(jt + 1) * 128],
                        hqT[p0:p0 + 64, dm:dm + 1, ic * 512:(ic + 1) * 512],
                    )
                    et = epool.tile([128, 512], bf16, tag="et")
                    nc.scalar.activation(et[:], sps[:], AF.Exp)
                    ets.append(et)
                cps = cps_p.tile([65, 512], f32, tag="c")
                for jt in range(JT):
                    nc.tensor.matmul(
                        cps[:],
                        hv[:, jt:jt + 1, h:h + 1, :],
                        ets[jt][:],
                        start=(jt == 0),
                        stop=(jt == JT - 1),
                    )
                rec = small.tile([1, 512], f32, tag="rec")
                nc.vector.reciprocal(rec[:], cps[64:65, :])
                bb = small.tile([64, 512], f32, tag="bb")
                nc.gpsimd.partition_broadcast(bb[:], rec[:])
                nc.vector.tensor_mul(ctxT[p0:p0 + 64, dm:dm + 1, :],
                                     cps[0:64, :], bb[:])

            for it2 in range(4):
                for ncol in range(2):
                    ops = mmps.tile([128, 512], f32, tag="mm")
                    for dm in range(DM):
                        nc.tensor.matmul(
                            ops[:],
                            ctxT[:, dm:dm + 1, it2 * 128:(it2 + 1) * 128],
                            woT[:, dm:dm + 1, ncol * 512:(ncol + 1) * 512],
                            start=(dm == 0),
                            stop=(dm == DM - 1),
                        )
                    osb = opool.tile([128, 512], f32, tag="osb")
                    nc.vector.tensor_copy(osb[:], ops[:])
                    nc.sync.dma_start(
                        out=out_d[ic * 512 + it2 * 128: ic * 512 + (it2 + 1) * 128,
                                  ncol * 512:(ncol + 1) * 512],
                        in_=osb[:],
                    )

    nc.compile()
    return nc


def _get_nc():
    if "nc" not in _CACHE:
        _CACHE["nc"] = _build()
    return _CACHE["nc"]


def _prep_inputs(q, k, v, Wq, bq, Wk, bk, Wv, bv, Wo, bo):
    import ml_dtypes
    c = np.ascontiguousarray
    f = np.float32
    bf = ml_dtypes.bfloat16
    # fold the attention scaling into the Q projection
    WqT = c((Wq.astype(f) * SC).T)           # [HID, HID]
    WkT = c(Wk.astype(f).T)
    WvT = c(Wv.astype(f).T)
    bq_s = (bq.astype(f) * SC)
    in_maps = []
    for b in range(B):
        qT = c(q[b].astype(f).T)             # [HID, L]
        kT = c(k[b].astype(f).T)
        vT = c(v[b].astype(f).T)
        for g in range(G):
            s = slice(g * GD, (g + 1) * GD)
            in_maps.append({
                "qT": qT, "kT": kT, "vT": vT,
                "wqT": c(WqT[:, s]),
                "wkT": c(WkT[:, s]),
                "wvT": c(WvT[:, s]),
                "woT": c(Wo[:, s].astype(f).T.reshape(DM, 128, HID)
                         .transpose(1, 0, 2).astype(bf)),
                "bq": c(bq_s[s].reshape(DM, 128).T),
                "bk": c(bk.astype(f)[s].reshape(DM, 128).T),
                "bv": c(bv.astype(f)[s].reshape(1, GD)),
                "ones": np.ones((1, 128), f),
            })
    return in_maps


def run(in_maps, trace=False, **kw):
    from concourse.bass_utils import run_bass_kernel_spmd
    nc = _get_nc()
    return run_bass_kernel_spmd(nc, in_maps, core_ids=list(range(8)),
                                trace=trace, **kw)


def kernel(q, k, v, Wq, bq, Wk, bk, Wv, bv, Wo, bo):
    in_maps = _prep_inputs(q, k, v, Wq, bq, Wk, bk, Wv, bv, Wo, bo)
    res = run(in_maps)
    out = np.zeros((B, L, HID), np.float32)
    for core, rm in enumerate(res.results):
        out[core // G] += rm["out"]
    out += bo.astype(np.float32)
    return out
